# revision 31
# baseline (speedup 1.0000x reference)
"""Trainium2 Bass kernel for nn_BCE_Loss (retrieval_knn).

Distributed strategy (8 NeuronCores, SPMD):
  - Host: L2-normalize rows in f32, quantize to the integer grid
    a = clip(round(64*x_hat), -16, 16) — every value and every product is
    exact in fp8 e4m3, so all PSUM partial sums are exact integers. Lay out
    a^T chunk-major [8 chunks, 128 D-partitions, 4 D-subtiles, 1024 rows];
    each core's input is chunk-rotated so its own rows form chunk 0.
  - Device per core: the [1024, 8192] raw-similarity stripe is computed as
    64 PSUM tiles [128, 1024] via fp8 DoubleRow matmuls (4 per tile,
    K=256 each). A 5th DoubleRow matmul per accumulation group multiplies
    constant iota tables (three e4m3-exact 4-bit pieces of the local column
    scaled by 2^-13) against an all-ones/2^-9 stationary, so PSUM comes out
    PRE-PACKED: p = S + col*2^-13 with S = <a_i, a_j> an integer — exact in
    f32 for |S| < 2^10 (true for all but ~1 pair in 33M). The diagonal
    (self-match) mask is ALSO a matmul: 128*I x (-48*I) subtracts 6144 on
    the static diagonal inside the same accumulation group. DVE then takes
    the per-1024-column-block top-8 with a single f32 max8 pass DIRECTLY
    from PSUM (value and column ride in one float) — the only cross-engine
    handoff in the main loop. No ACT/GPSIMD work, no on-device merge: the
    64 packed candidates per row go straight to DRAM.
  - Host: decode (S = floor(p), col = frac*8192 + 1024*block), divide by
    the exact norms of the quantized vectors, take top-k by corrected
    cosine, gather labels, BCE (tiny numpy work).

Validated in sim at rel err ~3.2e-3 vs the jax reference (tolerance 2e-2):
integer quantization adds ~0.004 cosine noise; top-k boundary swaps and the
winner's-curse on returned values dominate the error. Per-block top-8 union
misses are ~1e-3 of rows.

Measured per-op (this container, chained-slope method): DR matmul ~254ns
(N=512), DVE f32 max8 [128,1024] ~1.6-1.74us, ACT evac ~1.46us (unused),
Pool tensor_tensor ~2.9us (unused). Engine totals per core: PE ~90us
(6-7 matmuls/tile incl pack+mask), DVE ~111us (64 scans). The single
PE->DVE handoff pipeline overlaps almost fully: measured body time
~118us vs the 181us staged baseline (same slope methodology), with input
DMA (4MB fp8, 2 HWDGE queues) hidden under compute.
"""

from contextlib import ExitStack

import numpy as np
import ml_dtypes

import concourse.bass as bass
import concourse.mybir as mybir
import concourse.tile as tile
from concourse.bass import ts
from concourse.bass_utils import run_bass_kernel_spmd
from concourse.vector_clock import ScopedClock, VectorClock

F32 = mybir.dt.float32
FP8 = mybir.dt.float8e4
U32 = mybir.dt.uint32
I32 = mybir.dt.int32
AF = mybir.ActivationFunctionType
ALU = mybir.AluOpType
DR = mybir.MatmulPerfMode.DoubleRow

B, D = 8192, 512
M = 8              # cores
BL = B // M        # 1024 rows per core
NRT = BL // 128    # 8 row tiles per core
NEG = -3.0e38
EPS = 1e-12


# ---------------------------------------------------------------------------
# Environment workarounds: this container's walrus accepts at most ONE sem
# wait per instruction, and its runtime crashes on the explicit EventSemaphore
# butterfly barrier TileContext emits at its tail.
# ---------------------------------------------------------------------------

def _patched_drain_and_barrier(self, tick_clock, wait_clock):
    nc = self.nc
    vc = tick_clock.global_clock
    n = len(vc)
    for p in range(n):
        t = vc[p]
        if t > 0:
            pvc = VectorClock([0] * n)
            pvc.require_at_least(p, t)
            nop = nc.sync.nop()
            wait_clock.add_sem_waits(nop.ins, ScopedClock({None: pvc}))
    nc.sync.drain()
    nc._nrt_pseudo_barrier()
    assert self.sems is not None
    popped = nc._tile_sem_poison_stack.pop()
    assert popped is self._sem_poison
    nc.clear_and_free_semaphores(list(self.sems.allocated().values()))
    nc._nrt_pseudo_barrier()


tile.TileContext._drain_and_barrier = _patched_drain_and_barrier


def _split_multi_waits(nc):
    import bass_rust

    for f in nc.m.functions:
        for bb in f.blocks:
            out = []
            changed = False
            for ins in bb.instructions:
                si = ins.sync_info
                waits = list(si.on_wait) if si is not None else []
                if len(waits) > 1:
                    changed = True
                    for w in waits[:-1]:
                        nop = mybir.InstNoOp(
                            name=f"I-wsplit-{nc.next_id()}", ins=[], outs=[]
                        )
                        nop.engine = ins.engine
                        nop.sync_info = bass_rust.SyncInfo(on_wait=[w], on_update=[])
                        out.append(nop)
                    ins.sync_info = bass_rust.SyncInfo(
                        on_wait=[waits[-1]], on_update=list(si.on_update)
                    )
                out.append(ins)
            if changed:
                bb.instructions = out


# ---------------------------------------------------------------------------
# Kernel build
# ---------------------------------------------------------------------------

def build_nc(repeat=1):
    nc = bass.Bass(num_devices=M)
    xq = nc.declare_dram_parameter("xq", [8, 128, 4, BL], FP8, isOutput=False)
    it = nc.declare_dram_parameter("it", [2, 2, BL], FP8, isOutput=False)
    iwd = nc.declare_dram_parameter("iw", [2, 2, 128], FP8, isOutput=False)
    out = nc.declare_dram_parameter("out", [BL, 64], F32, isOutput=True)
    for _rep in range(repeat):
        _build_body(nc, xq, it, iwd, out)
    _split_multi_waits(nc)
    return nc


def _build_body(nc, xq_dram, it_dram, iw_dram, out):
    with tile.TileContext(nc) as tc, ExitStack() as octx:
        cpool = octx.enter_context(tc.tile_pool(name="const", bufs=1))
        # fp8 identity pair whose product subtracts 6144 on the diagonal
        # (S_diag ~ 4139 vs off-diag |S| <~ 1500): the mask rides as one
        # extra matmul in the accumulation group, costing no DVE/ACT time
        itl = cpool.tile([128, 128], I32)
        nc.gpsimd.iota(itl[:], pattern=[[1, 128]], base=0,
                       channel_multiplier=-1)
        id128 = cpool.tile([128, 128], FP8)
        nc.vector.tensor_scalar(id128[:], in0=itl[:], scalar1=0,
                                scalar2=128.0, op0=ALU.is_equal,
                                op1=ALU.mult)
        idm48 = cpool.tile([128, 128], FP8)
        nc.vector.tensor_scalar(idm48[:], in0=itl[:], scalar1=0,
                                scalar2=-48.0, op0=ALU.is_equal,
                                op1=ALU.mult)
        # stationary operand for the iota-pack matmul (host-supplied: engines
        # cannot write at a partition offset, DMA can)
        iw = cpool.tile([2, 2, 128], FP8)

        # a^T chunks: [128 D-partitions, 4 D-subtiles, 1024 rows] fp8 each
        xt_pool = octx.enter_context(tc.tile_pool(name="xt", bufs=1))
        xt = [
            xt_pool.tile([128, 4, BL], FP8, tag=f"xt_{ch}", name=f"xt_{ch}")
            for ch in range(8)
        ]
        itt = xt_pool.tile([2, 2, BL], FP8, tag="itt", name="itt")

        mm = octx.enter_context(tc.tile_pool(name="mm", bufs=4, space="PSUM"))
        cand = octx.enter_context(tc.tile_pool(name="cand", bufs=1))

        # two HWDGE queues (qSP ~85 GB/s each); low chunks land first, and
        # chunks 0/1 are split across both queues so compute starts sooner
        nc.sync.dma_start(itt[:], it_dram[:])
        nc.scalar.dma_start(iw[:], iw_dram[:])
        for ch in range(2):
            nc.sync.dma_start(xt[ch][:, :, 0:512], xq_dram[ch][:, :, 0:512])
            nc.scalar.dma_start(xt[ch][:, :, 512:1024],
                                xq_dram[ch][:, :, 512:1024])
        for ch in range(2, 8):
            eng = nc.sync if ch % 2 == 0 else nc.scalar
            eng.dma_start(xt[ch][:], xq_dram[ch])

        vals = [
            cand.tile([128, 64], F32, tag=f"VALS{m}", name=f"VALS{m}")
            for m in range(NRT)
        ]

        def do_block(pair, m):
            cbs = (2 * pair, 2 * pair + 1)
            pss = [
                mm.tile([128, 1024], F32, tag="ps", name=f"ps_{m}_{cb}")
                for cb in cbs
            ]
            # g-outer ordering: one weight load serves 4 matmuls
            for g in range(2):
                lhsT = xt[0][:, 2 * g:2 * g + 2, ts(m, 128)]
                for j, cb in enumerate(cbs):
                    for h in range(2):
                        nc.tensor.matmul(
                            pss[j][:, ts(h, 512)], lhsT,
                            xt[cb][:, 2 * g:2 * g + 2, ts(h, 512)],
                            start=(g == 0), stop=False,
                            perf_mode=DR,
                        )
            # pack pass: adds col*2^-13 (three e4m3-exact pieces) into PSUM;
            # for cb0 a diagonal-mask matmul (-6144*I) joins the group
            for j, cb in enumerate(cbs):
                if cb == 0:
                    o = m * 128
                    h0 = o // 512
                    nc.tensor.matmul(
                        pss[j][:, o:o + 128], id128[:],
                        idm48[:],
                        start=False, stop=False,
                    )
                for h in range(2):
                    nc.tensor.matmul(
                        pss[j][:, ts(h, 512)], iw[:, :, :],
                        itt[:, :, ts(h, 512)],
                        start=False, stop=True, perf_mode=DR,
                    )
            for j, cb in enumerate(cbs):
                # one f32 max8 pass directly on pre-packed PSUM gets
                # value+column together (no evacuation stage at all)
                nc.vector.max(vals[m][:, cb * 8:cb * 8 + 8], pss[j][:])

        # no on-device merge: the host selects top-k from the 64 packed
        # candidates per row (trivial numpy work, keeps DVE scan-only)
        for pair in range(4):
            for m in range(NRT):
                do_block(pair, m)
                if pair == 3:
                    nc.sync.dma_start(out[ts(m, 128), :], vals[m][:])


_NC = None


def _get_nc():
    global _NC
    if _NC is None:
        _NC = build_nc()
    return _NC


def make_iota_table():
    """it[k, o, n]: three e4m3-exact pieces of n*2^-13, n in [0, 1024).

    Pairs with iw = [[1.0, 2^-9], [2^-9, 0]]:
      (0,0): 1.0  * (n>>8)*2^-5      = c2*2^8*2^-13
      (0,1): 2^-9 * ((n>>4)&15)*1.0  = c1*2^4*2^-13
      (1,0): 2^-9 * (n&15)*2^-4      = c0*2^-13
      (1,1): 0    * 0
    """
    n = np.arange(BL, dtype=np.int64)
    t = np.zeros((2, 2, BL), np.float32)
    t[0, 0] = (n >> 8).astype(np.float32) * 2.0 ** -5
    t[0, 1] = ((n >> 4) & 15).astype(np.float32)
    t[1, 0] = (n & 15).astype(np.float32) * 2.0 ** -4
    t8 = t.astype(ml_dtypes.float8_e4m3)
    assert np.array_equal(t8.astype(np.float32), t), "iota table not exact"
    return t8


def make_iota_weights():
    w = np.zeros((2, 2, 128), np.float32)
    w[0, 0] = 1.0
    w[0, 1] = 2.0 ** -9
    w[1, 0] = 2.0 ** -9
    return w.astype(ml_dtypes.float8_e4m3)


def quantize(x32):
    """Normalize rows, quantize to integers in [-16, 16] (exact in e4m3)."""
    n = np.sqrt(np.einsum("ij,ij->i", x32, x32, dtype=np.float64))
    n = np.maximum(n, EPS).astype(np.float32)
    xh = x32 / n[:, None]
    a = np.clip(np.round(xh * 64.0), -16, 16).astype(np.float32)
    return a


def prep_inputs(x32, a=None):
    """Host prep: quantize, fp8 cast, transpose chunk-major, rotate."""
    if a is None:
        a = quantize(x32)
    x8 = a.astype(ml_dtypes.float8_e4m3)
    it = make_iota_table()
    iw = make_iota_weights()
    # C[ch, p, d4, t] = x8[ch*1024 + t, d4*128 + p]
    C = np.ascontiguousarray(x8.reshape(8, BL, 4, 128).transpose(0, 3, 2, 1))
    return [
        {"xq": np.ascontiguousarray(C[(np.arange(8) + c) % 8]),
         "it": it, "iw": iw}
        for c in range(M)
    ]


def run_device(x32, trace=False, **kwargs):
    """Run the SPMD kernel; returns (pv [B, 24] f32, BassKernelResults)."""
    nc = _get_nc()
    in_maps = prep_inputs(x32)
    res = run_bass_kernel_spmd(nc, in_maps, core_ids=list(range(M)),
                               trace=trace, **kwargs)
    pv = np.concatenate([res.results[c]["out"] for c in range(M)], axis=0)
    return pv, res


def decode_loss(pv, labels, k, a):
    """Decode the 64 packed block-candidates per row -> BCE loss.

    pv[:, i] (slot i = 8*cb + rank) is S + n*2^-13 with S = <a_i, a_j> an
    integer and n the column within 1024-column block cb. The device ranks
    by the raw quantized dot; the host adds block offsets, divides by the
    exact norms of the quantized vectors, and takes top-k by cosine.
    """
    p64 = pv.astype(np.float64)
    S = np.floor(p64)
    n = np.round((p64 - S) * 8192.0).astype(np.int64) % 1024
    blk = (np.arange(64) // 8) * 1024
    col = n + blk[None, :]                            # local column
    core = np.arange(B) // BL                         # global row -> core
    gidx = (col + (core * BL)[:, None]) % B           # local -> global column
    nq = np.sqrt((a.astype(np.float64) ** 2).sum(1))
    vhat = S / (nq[:, None] * nq[gidx])               # corrected cosine
    o2 = np.argsort(-vhat, axis=1, kind="stable")[:, :k]
    vk = np.take_along_axis(vhat, o2, axis=1)
    ck = np.take_along_axis(gidx, o2, axis=1)
    preds = (vk + 1.0) * 0.5
    t = (labels[ck] == labels[:, None]).astype(np.float64)
    logp = np.maximum(np.log(preds), -100.0)
    log1mp = np.maximum(np.log1p(-preds), -100.0)
    loss = -(t * logp + (1.0 - t) * log1mp)
    return np.float32(loss.mean())


def kernel(batch, labels, k):
    k = int(k)
    assert 0 < k <= 24, f"kernel supports k <= 24, got {k}"
    x32 = np.asarray(batch, dtype=np.float32)
    assert x32.shape == (B, D)
    labels = np.asarray(labels)
    a = quantize(x32)
    nc = _get_nc()
    in_maps = prep_inputs(x32, a)
    res = run_bass_kernel_spmd(nc, in_maps, core_ids=list(range(M)))
    pv = np.concatenate([res.results[c]["out"] for c in range(M)], axis=0)
    return decode_loss(pv, labels, k, a)


# revision 32
# speedup vs baseline: 1.4999x; 1.4999x over previous
"""Trainium2 Bass kernel for nn_BCE_Loss (retrieval_knn).

Distributed strategy (8 NeuronCores, SPMD):
  - Host: L2-normalize rows in f32, quantize to the integer grid
    a = clip(round(64*x_hat), -16, 16) — every value and every product is
    exact in fp8 e4m3, so all PSUM partial sums are exact integers. Lay out
    a^T chunk-major [8 chunks, 128 D-partitions, 4 D-subtiles, 1024 rows];
    each core's input is chunk-rotated so its own rows form chunk 0.
  - Device per core: the [1024, 8192] raw-similarity stripe is computed as
    64 PSUM tiles [128, 1024] via fp8 DoubleRow matmuls (4 per tile,
    K=256 each). A 5th DoubleRow matmul per accumulation group multiplies
    constant iota tables (three e4m3-exact 4-bit pieces of the local column
    scaled by 2^-13) against an all-ones/2^-9 stationary, so PSUM comes out
    PRE-PACKED: p = S + col*2^-13 with S = <a_i, a_j> an integer — exact in
    f32 for |S| < 2^10 (true for all but ~1 pair in 33M). The diagonal
    (self-match) mask is ALSO a matmul: 128*I x (-48*I) subtracts 6144 on
    the static diagonal inside the same accumulation group. DVE then takes
    the per-1024-column-block top-8 with a single f32 max8 pass DIRECTLY
    from PSUM (value and column ride in one float) — the only cross-engine
    handoff in the main loop. No ACT/GPSIMD work, no on-device merge: the
    64 packed candidates per row go straight to DRAM.
  - Host: decode (S = floor(p), col = frac*8192 + 1024*block), divide by
    the exact norms of the quantized vectors, take top-k by corrected
    cosine, gather labels, BCE (tiny numpy work).

Validated in sim at rel err ~3.2e-3 vs the jax reference (tolerance 2e-2):
integer quantization adds ~0.004 cosine noise; top-k boundary swaps and the
winner's-curse on returned values dominate the error. Per-block top-8 union
misses are ~1e-3 of rows.

Measured per-op (this container, chained-slope method): DR matmul ~254ns
(N=512), DVE f32 max8 [128,1024] ~1.6-1.74us, ACT evac ~1.46us (unused),
Pool tensor_tensor ~2.9us (unused). Engine totals per core: PE ~90us
(6-7 matmuls/tile incl pack+mask), DVE ~111us (64 scans). The single
PE->DVE handoff pipeline overlaps almost fully: measured body time
~118us vs the 181us staged baseline (same slope methodology), with input
DMA (4MB fp8, 2 HWDGE queues) hidden under compute.
"""

from contextlib import ExitStack

import numpy as np
import ml_dtypes

import concourse.bass as bass
import concourse.mybir as mybir
import concourse.tile as tile
from concourse.bass import ts
from concourse.bass_utils import run_bass_kernel_spmd
from concourse.vector_clock import ScopedClock, VectorClock

F32 = mybir.dt.float32
FP8 = mybir.dt.float8e4
U32 = mybir.dt.uint32
I32 = mybir.dt.int32
AF = mybir.ActivationFunctionType
ALU = mybir.AluOpType
DR = mybir.MatmulPerfMode.DoubleRow

B, D = 8192, 512
M = 8              # cores
BL = B // M        # 1024 rows per core
NRT = BL // 128    # 8 row tiles per core
NEG = -3.0e38
EPS = 1e-12


# ---------------------------------------------------------------------------
# Environment workarounds: this container's walrus accepts at most ONE sem
# wait per instruction, and its runtime crashes on the explicit EventSemaphore
# butterfly barrier TileContext emits at its tail.
# ---------------------------------------------------------------------------

def _patched_drain_and_barrier(self, tick_clock, wait_clock):
    nc = self.nc
    vc = tick_clock.global_clock
    n = len(vc)
    for p in range(n):
        t = vc[p]
        if t > 0:
            pvc = VectorClock([0] * n)
            pvc.require_at_least(p, t)
            nop = nc.sync.nop()
            wait_clock.add_sem_waits(nop.ins, ScopedClock({None: pvc}))
    nc.sync.drain()
    nc._nrt_pseudo_barrier()
    assert self.sems is not None
    popped = nc._tile_sem_poison_stack.pop()
    assert popped is self._sem_poison
    nc.clear_and_free_semaphores(list(self.sems.allocated().values()))
    nc._nrt_pseudo_barrier()


tile.TileContext._drain_and_barrier = _patched_drain_and_barrier


def _split_multi_waits(nc):
    import bass_rust

    for f in nc.m.functions:
        for bb in f.blocks:
            out = []
            changed = False
            for ins in bb.instructions:
                si = ins.sync_info
                waits = list(si.on_wait) if si is not None else []
                if len(waits) > 1:
                    changed = True
                    for w in waits[:-1]:
                        nop = mybir.InstNoOp(
                            name=f"I-wsplit-{nc.next_id()}", ins=[], outs=[]
                        )
                        nop.engine = ins.engine
                        nop.sync_info = bass_rust.SyncInfo(on_wait=[w], on_update=[])
                        out.append(nop)
                    ins.sync_info = bass_rust.SyncInfo(
                        on_wait=[waits[-1]], on_update=list(si.on_update)
                    )
                out.append(ins)
            if changed:
                bb.instructions = out


# ---------------------------------------------------------------------------
# Kernel build
# ---------------------------------------------------------------------------

def build_nc(repeat=1):
    nc = bass.Bass(num_devices=M)
    xq = nc.declare_dram_parameter("xq", [8, 128, 4, BL], FP8, isOutput=False)
    it = nc.declare_dram_parameter("it", [2, 2, BL], FP8, isOutput=False)
    iwd = nc.declare_dram_parameter("iw", [2, 2, 128], FP8, isOutput=False)
    out = nc.declare_dram_parameter("out", [BL, 64], F32, isOutput=True)
    for _rep in range(repeat):
        _build_body(nc, xq, it, iwd, out)
    _split_multi_waits(nc)
    return nc


def _build_body(nc, xq_dram, it_dram, iw_dram, out):
    with tile.TileContext(nc) as tc, ExitStack() as octx:
        cpool = octx.enter_context(tc.tile_pool(name="const", bufs=1))
        # fp8 identity pair whose product subtracts 6144 on the diagonal
        # (S_diag ~ 4139 vs off-diag |S| <~ 1500): the mask rides as one
        # extra matmul in the accumulation group, costing no DVE/ACT time
        itl = cpool.tile([128, 128], I32)
        nc.gpsimd.iota(itl[:], pattern=[[1, 128]], base=0,
                       channel_multiplier=-1)
        id128 = cpool.tile([128, 128], FP8)
        nc.vector.tensor_scalar(id128[:], in0=itl[:], scalar1=0,
                                scalar2=128.0, op0=ALU.is_equal,
                                op1=ALU.mult)
        idm48 = cpool.tile([128, 128], FP8)
        nc.vector.tensor_scalar(idm48[:], in0=itl[:], scalar1=0,
                                scalar2=-48.0, op0=ALU.is_equal,
                                op1=ALU.mult)
        # stationary operand for the iota-pack matmul (host-supplied: engines
        # cannot write at a partition offset, DMA can)
        iw = cpool.tile([2, 2, 128], FP8)

        # a^T chunks: [128 D-partitions, 4 D-subtiles, 1024 rows] fp8 each
        xt_pool = octx.enter_context(tc.tile_pool(name="xt", bufs=1))
        xt = [
            xt_pool.tile([128, 4, BL], FP8, tag=f"xt_{ch}", name=f"xt_{ch}")
            for ch in range(8)
        ]
        itt = xt_pool.tile([2, 2, BL], FP8, tag="itt", name="itt")

        mm = octx.enter_context(tc.tile_pool(name="mm", bufs=4, space="PSUM"))
        cand = octx.enter_context(tc.tile_pool(name="cand", bufs=1))

        # two HWDGE queues (qSP ~85 GB/s each); low chunks land first
        nc.sync.dma_start(itt[:], it_dram[:])
        nc.scalar.dma_start(iw[:], iw_dram[:])
        for ch in range(8):
            eng = nc.sync if ch % 2 == 0 else nc.scalar
            eng.dma_start(xt[ch][:], xq_dram[ch])

        vals = [
            cand.tile([128, 64], F32, tag=f"VALS{m}", name=f"VALS{m}")
            for m in range(NRT)
        ]

        def do_block(pair, m):
            cbs = (2 * pair, 2 * pair + 1)
            pss = [
                mm.tile([128, 1024], F32, tag="ps", name=f"ps_{m}_{cb}")
                for cb in cbs
            ]
            # g-outer ordering: one weight load serves 4 matmuls
            for g in range(2):
                lhsT = xt[0][:, 2 * g:2 * g + 2, ts(m, 128)]
                for j, cb in enumerate(cbs):
                    for h in range(2):
                        nc.tensor.matmul(
                            pss[j][:, ts(h, 512)], lhsT,
                            xt[cb][:, 2 * g:2 * g + 2, ts(h, 512)],
                            start=(g == 0), stop=False,
                            perf_mode=DR,
                        )
            # pack pass: adds col*2^-13 (three e4m3-exact pieces) into PSUM;
            # for cb0 a diagonal-mask matmul (-6144*I) joins the group
            for j, cb in enumerate(cbs):
                if cb == 0:
                    o = m * 128
                    h0 = o // 512
                    nc.tensor.matmul(
                        pss[j][:, o:o + 128], id128[:],
                        idm48[:],
                        start=False, stop=False,
                    )
                for h in range(2):
                    nc.tensor.matmul(
                        pss[j][:, ts(h, 512)], iw[:, :, :],
                        itt[:, :, ts(h, 512)],
                        start=False, stop=True, perf_mode=DR,
                    )
            for j, cb in enumerate(cbs):
                # one f32 max8 pass directly on pre-packed PSUM gets
                # value+column together (no evacuation stage at all)
                nc.vector.max(vals[m][:, cb * 8:cb * 8 + 8], pss[j][:])

        # no on-device merge: the host selects top-k from the 64 packed
        # candidates per row (trivial numpy work, keeps DVE scan-only)
        for pair in range(4):
            for m in range(NRT):
                do_block(pair, m)
                if pair == 3:
                    nc.sync.dma_start(out[ts(m, 128), :], vals[m][:])


_NC = None


def _get_nc():
    global _NC
    if _NC is None:
        _NC = build_nc()
    return _NC


def make_iota_table():
    """it[k, o, n]: three e4m3-exact pieces of n*2^-13, n in [0, 1024).

    Pairs with iw = [[1.0, 2^-9], [2^-9, 0]]:
      (0,0): 1.0  * (n>>8)*2^-5      = c2*2^8*2^-13
      (0,1): 2^-9 * ((n>>4)&15)*1.0  = c1*2^4*2^-13
      (1,0): 2^-9 * (n&15)*2^-4      = c0*2^-13
      (1,1): 0    * 0
    """
    n = np.arange(BL, dtype=np.int64)
    t = np.zeros((2, 2, BL), np.float32)
    t[0, 0] = (n >> 8).astype(np.float32) * 2.0 ** -5
    t[0, 1] = ((n >> 4) & 15).astype(np.float32)
    t[1, 0] = (n & 15).astype(np.float32) * 2.0 ** -4
    t8 = t.astype(ml_dtypes.float8_e4m3)
    assert np.array_equal(t8.astype(np.float32), t), "iota table not exact"
    return t8


def make_iota_weights():
    w = np.zeros((2, 2, 128), np.float32)
    w[0, 0] = 1.0
    w[0, 1] = 2.0 ** -9
    w[1, 0] = 2.0 ** -9
    return w.astype(ml_dtypes.float8_e4m3)


def quantize(x32):
    """Normalize rows, quantize to integers in [-16, 16] (exact in e4m3)."""
    n = np.sqrt(np.einsum("ij,ij->i", x32, x32, dtype=np.float64))
    n = np.maximum(n, EPS).astype(np.float32)
    xh = x32 / n[:, None]
    a = np.clip(np.round(xh * 64.0), -16, 16).astype(np.float32)
    return a


def prep_inputs(x32, a=None):
    """Host prep: quantize, fp8 cast, transpose chunk-major, rotate."""
    if a is None:
        a = quantize(x32)
    x8 = a.astype(ml_dtypes.float8_e4m3)
    it = make_iota_table()
    iw = make_iota_weights()
    # C[ch, p, d4, t] = x8[ch*1024 + t, d4*128 + p]
    C = np.ascontiguousarray(x8.reshape(8, BL, 4, 128).transpose(0, 3, 2, 1))
    return [
        {"xq": np.ascontiguousarray(C[(np.arange(8) + c) % 8]),
         "it": it, "iw": iw}
        for c in range(M)
    ]


def run_device(x32, trace=False, **kwargs):
    """Run the SPMD kernel; returns (pv [B, 24] f32, BassKernelResults)."""
    nc = _get_nc()
    in_maps = prep_inputs(x32)
    res = run_bass_kernel_spmd(nc, in_maps, core_ids=list(range(M)),
                               trace=trace, **kwargs)
    pv = np.concatenate([res.results[c]["out"] for c in range(M)], axis=0)
    return pv, res


def decode_loss(pv, labels, k, a):
    """Decode the 64 packed block-candidates per row -> BCE loss.

    pv[:, i] (slot i = 8*cb + rank) is S + n*2^-13 with S = <a_i, a_j> an
    integer and n the column within 1024-column block cb. The device ranks
    by the raw quantized dot; the host adds block offsets, divides by the
    exact norms of the quantized vectors, and takes top-k by cosine.
    """
    p64 = pv.astype(np.float64)
    S = np.floor(p64)
    n = np.round((p64 - S) * 8192.0).astype(np.int64) % 1024
    blk = (np.arange(64) // 8) * 1024
    col = n + blk[None, :]                            # local column
    core = np.arange(B) // BL                         # global row -> core
    gidx = (col + (core * BL)[:, None]) % B           # local -> global column
    nq = np.sqrt((a.astype(np.float64) ** 2).sum(1))
    vhat = S / (nq[:, None] * nq[gidx])               # corrected cosine
    o2 = np.argsort(-vhat, axis=1, kind="stable")[:, :k]
    vk = np.take_along_axis(vhat, o2, axis=1)
    ck = np.take_along_axis(gidx, o2, axis=1)
    preds = (vk + 1.0) * 0.5
    t = (labels[ck] == labels[:, None]).astype(np.float64)
    logp = np.maximum(np.log(preds), -100.0)
    log1mp = np.maximum(np.log1p(-preds), -100.0)
    loss = -(t * logp + (1.0 - t) * log1mp)
    return np.float32(loss.mean())


def kernel(batch, labels, k):
    k = int(k)
    assert 0 < k <= 24, f"kernel supports k <= 24, got {k}"
    x32 = np.asarray(batch, dtype=np.float32)
    assert x32.shape == (B, D)
    labels = np.asarray(labels)
    a = quantize(x32)
    nc = _get_nc()
    in_maps = prep_inputs(x32, a)
    res = run_bass_kernel_spmd(nc, in_maps, core_ids=list(range(M)))
    pv = np.concatenate([res.results[c]["out"] for c in range(M)], axis=0)
    return decode_loss(pv, labels, k, a)
